# revision 2
# baseline (speedup 1.0000x reference)
"""DiscreteOptionActor Trainium2 kernel (v2).

Computes, for each sample b, logits = MLP_{option[b]}(obs[b]) where each of the
16 options has its own 3-layer MLP (128 -> 256 -> 256 -> 18, ReLU).

Strategy (MoE routing, option-parallel):
  - Host groups samples by option (argsort); the 16 options are paired
    largest-with-smallest and one pair is assigned per core, so per-core
    column counts are balanced. Only the selected option's trunk is
    computed (16x less compute than the dense reference).
  - Per (core, slot) the gathered rows are padded to C0/C1 columns
    (multiples of 128, global maxima over cores so the SPMD program has
    one shape) and stored feature-major [128, C] in fp16.
  - Device: L1/L2 run fp16 matmuls with 512-col moving chunks into a
    4-slot PSUM rotation; bias+ReLU drains alternate ACT/DVE writing
    fp16. L3 (M=18) uses 3x PE column tiling (tile_position=(0,32j)):
    three 18-row output strips computed concurrently in one pass over
    each third of the columns.
  - Warm-up: dummy matmuls bridge the DMA latency window so the PE HAM
    clock-gate reaches 8/8 (2.4 GHz) before real work, with no idle gap.
  - DMA: three rings (sync HWDGE, scalar HWDGE, gpsimd SWDGE) stream xt
    and weights; first xt chunk + W1 lead the sync ring so L1 starts
    ~9us into the NEFF.
  - Host scatters results back to original row order and adds b3.
"""

import numpy as np

B, OBS, OPT, H1, H2, A = 65536, 128, 16, 256, 256, 18
NCORES = 8
OPC = 2  # options per core

_CACHE = {}

N_WARM_BIG = 5   # 512-col dummy matmuls
N_WARM_SMALL = 6  # 128-col dummy matmuls


def _halves(nb):
    out = []
    h = 0
    while h < nb:
        w = min(512, nb - h)
        out.append((h, w))
        h += w
    return out


def _pairs_for(C):
    """L1/L2 column blocks: 512,512 first (early start), then 1024s."""
    out = []
    st = 0
    for nb in (512, 512):
        if st < C:
            nb = min(nb, C - st)
            out.append((st, nb))
            st += nb
    while st < C:
        nb = min(1024, C - st)
        out.append((st, nb))
        st += nb
    return out


def _l3_blocks_for(P):
    out = []
    st = 0
    while st < P:
        nb = min(1024, P - st)
        out.append((st, nb))
        st += nb
    return out


def _xchunks_for(C, lead):
    """3 DMA chunks: [0,lead) on sync, then scalar, then gpsimd."""
    q1 = min(lead, C)
    rem = C - q1
    q2 = q1 + (rem + 1) // 2
    return [(0, q1), (q1, q2 - q1), (q2, C - q2)]


def _build_v2(C0, C1):
    import concourse.bass as bass
    import concourse.bacc as bacc
    import concourse.mybir as mybir

    f32 = mybir.dt.float32
    f16 = mybir.dt.float16
    AF = mybir.ActivationFunctionType
    ALU = mybir.AluOpType

    Cs = [C0, C1]
    Ps = [(C + 2) // 3 for C in Cs]

    nc = bacc.Bacc(None, target_bir_lowering=False, debug=False)

    xt_p, w1_p, w2_p, w3_p, b1_p, b2_p, out_p = [], [], [], [], [], [], []
    for lo in range(OPC):
        C, P = Cs[lo], Ps[lo]
        xt_p.append(nc.declare_dram_parameter(f"xt{lo}", [1, OBS, C], f16, isOutput=False))
        w1_p.append(nc.declare_dram_parameter(f"w1_{lo}", [1, OBS, H1], f16, isOutput=False))
        w2_p.append(nc.declare_dram_parameter(f"w2_{lo}", [1, 128, 2, H2], f16, isOutput=False))
        w3_p.append(nc.declare_dram_parameter(f"w3_{lo}", [1, 128, 2, A], f16, isOutput=False))
        b1_p.append(nc.declare_dram_parameter(f"b1_{lo}", [1, 128, 2], f32, isOutput=False))
        b2_p.append(nc.declare_dram_parameter(f"b2_{lo}", [1, 128, 2], f32, isOutput=False))
        out_p.append(nc.declare_dram_parameter(f"out{lo}", [1, 96, P], f16, isOutput=True))

    # --- on-chip tensors ---
    xts = [nc.alloc_sbuf_tensor(f"xts{lo}", [OBS, Cs[lo]], f16) for lo in range(OPC)]
    w1s = [nc.alloc_sbuf_tensor(f"w1s{lo}", [OBS, H1], f16) for lo in range(OPC)]
    w2s = [nc.alloc_sbuf_tensor(f"w2s{lo}", [128, 2, H2], f16) for lo in range(OPC)]
    w3s = [nc.alloc_sbuf_tensor(f"w3s{lo}", [128, 2, A], f16) for lo in range(OPC)]
    b1s = [nc.alloc_sbuf_tensor(f"b1s{lo}", [128, 2], f32) for lo in range(OPC)]
    b2s = [nc.alloc_sbuf_tensor(f"b2s{lo}", [128, 2], f32) for lo in range(OPC)]
    h1s = [[nc.alloc_sbuf_tensor(f"h1_{lo}_{c}", [128, Cs[lo]], f16) for c in range(2)]
           for lo in range(OPC)]
    h2s = [[nc.alloc_sbuf_tensor(f"h2_{lo}_{m}", [128, Cs[lo]], f16) for m in range(2)]
           for lo in range(OPC)]
    osb = [nc.alloc_sbuf_tensor(f"osb{lo}", [96, Ps[lo]], f16) for lo in range(OPC)]
    dummy = nc.alloc_sbuf_tensor("warm_dummy", [128, 512], f16)
    dummy_o = nc.alloc_sbuf_tensor("warm_dummy_o", [128, 1], f32)

    pss = [nc.alloc_psum_tensor(f"ps{s}", [128, 1024], f32) for s in range(4)]

    # --- semaphores ---
    xchunks = [_xchunks_for(Cs[0], 512), _xchunks_for(Cs[1], 1024)]
    xsem = [[nc.alloc_semaphore(f"x{lo}_{ci}") for ci in range(3)] for lo in range(OPC)]
    wname = ["w1", "b1", "b2", "w2", "w3"]
    wsem = [{n: nc.alloc_semaphore(f"wt{lo}_{n}") for n in wname} for lo in range(OPC)]
    fd = [nc.alloc_semaphore(f"fd{s}") for s in range(4)]
    prog = {}
    for lo in range(OPC):
        for key in ("h1a", "h1v", "h2a", "h2v", "oa", "ov"):
            prog[(key, lo)] = nc.alloc_semaphore(f"{key}{lo}")
    od_sync = nc.alloc_semaphore("od_sync")
    od_gps = nc.alloc_semaphore("od_gps")

    # --- static schedule state ---
    pe_ops = []   # (waits, slot, mms, fdsem); mm = (out_lo, out_rows, h, w, lhs_fn, rhs_fn, start, stop, tile_pos)
    act_ops = []  # (waits, kind, slot, rows, dst_fn, bias_fn, sem)
    dve_ops = []
    sync_ops = []  # ("dma", waits, dst_fn, src_fn, sem, val)
    scalar_dma_ops = []
    gps_ops = []

    fill_count = [0, 0, 0, 0]
    slot_prev_drain = [None, None, None, None]
    fill_idx = 0
    prog_count = {k: 0 for k in prog}
    pe_last_wait = {}

    def pe_wait(waits, sem, val):
        key = sem.name if hasattr(sem, "name") else id(sem)
        if pe_last_wait.get(key, -1) < val:
            waits.append((sem, val))
            pe_last_wait[key] = val

    def emit_fill(data_waits, mms, drain_engine, drain_emit):
        nonlocal fill_idx
        s = fill_idx % 4
        fill_idx += 1
        waits = []
        if slot_prev_drain[s] is not None:
            sem, cnt = slot_prev_drain[s]
            pe_wait(waits, sem, cnt)
        for sem, val in data_waits:
            pe_wait(waits, sem, val)
        pe_ops.append((waits, s, mms, fd[s]))
        fill_count[s] += 1
        fd_thresh = fill_count[s]
        kind, rows, dst_fn, bias, psem_key, extra_waits = drain_emit
        sem = prog[psem_key]
        prog_count[psem_key] += 1
        cnt = prog_count[psem_key]
        drain_waits = [(fd[s], fd_thresh)] + extra_waits
        op = (drain_waits, kind, s, rows, dst_fn, bias, sem)
        if drain_engine == "act":
            act_ops.append(op)
        else:
            dve_ops.append(op)
        slot_prev_drain[s] = (sem, cnt)
        return cnt

    # --- DMA issue schedule ---
    def xdma(lo, ci):
        cst, cnb = xchunks[lo][ci]
        return ("dma", [],
                (lambda lo=lo, cst=cst, cnb=cnb: xts[lo].ap()[:, cst:cst + cnb]),
                (lambda lo=lo, cst=cst, cnb=cnb: xt_p[lo][0][:, cst:cst + cnb]),
                xsem[lo][ci], 16)

    def wdma(lo, n):
        src = {"w1": w1_p, "b1": b1_p, "b2": b2_p, "w2": w2_p, "w3": w3_p}[n]
        dst = {"w1": w1s, "b1": b1s, "b2": b2s, "w2": w2s, "w3": w3s}[n]
        return ("dma", [], (lambda lo=lo, dst=dst: dst[lo].ap()[:]),
                (lambda lo=lo, src=src: src[lo][0]), wsem[lo][n], 16)

    for lo in range(OPC):
        sync_ops.append(xdma(lo, 0))
        sync_ops.extend(wdma(lo, n) for n in wname)
        scalar_dma_ops.append(xdma(lo, 1))
        gps_ops.append(xdma(lo, 2))

    # --- fills ---
    pairs = [_pairs_for(C) for C in Cs]
    l3blocks = [_l3_blocks_for(P) for P in Ps]
    di = 0
    l1_thr = {}
    l2_thr = {}
    out_dma_counts = {"sync": 0, "gps": 0}

    def chunk_hi(lo, st, nb):
        hi = 0
        for ci, (cst, cnb) in enumerate(xchunks[lo]):
            if cst < st + nb:
                hi = ci
        return hi

    def emit_l1(lo, p):
        nonlocal di
        st, nb = pairs[lo][p]
        for c in range(2):
            data_waits = [(xsem[lo][ci], 16) for ci in range(chunk_hi(lo, st, nb) + 1)]
            data_waits.append((wsem[lo]["w1"], 16))
            mms = []
            for h, w in _halves(nb):
                mms.append((
                    0, 128, h, w,
                    (lambda lo=lo, c=c: w1s[lo].ap()[:, c * 128:(c + 1) * 128]),
                    (lambda lo=lo, st=st, h=h, w=w: xts[lo].ap()[:, st + h:st + h + w]),
                    True, True, None,
                ))
            eng = "act" if di % 2 == 0 else "dve"
            emit_fill(
                data_waits, mms, eng,
                ("relu", 128,
                 (lambda lo=lo, c=c, st=st, nb=nb: h1s[lo][c].ap()[:, st:st + nb]),
                 (lambda lo=lo, c=c: b1s[lo].ap()[:, c:c + 1]),
                 ("h1a" if eng == "act" else "h1v", lo),
                 [(wsem[lo]["b1"], 16)]),
            )
            di += 1
        l1_thr[(lo, p)] = (prog_count[("h1a", lo)], prog_count[("h1v", lo)])

    def emit_l2(lo, p):
        nonlocal di
        st, nb = pairs[lo][p]
        na, nv = l1_thr[(lo, p)]
        for m in range(2):
            data_waits = [(wsem[lo]["w2"], 16)]
            if na:
                data_waits.append((prog[("h1a", lo)], na))
            if nv:
                data_waits.append((prog[("h1v", lo)], nv))
            mms = []
            for h, w in _halves(nb):
                for k in range(2):
                    mms.append((
                        0, 128, h, w,
                        (lambda lo=lo, k=k, m=m: w2s[lo].ap()[:, k, m * 128:(m + 1) * 128]),
                        (lambda lo=lo, k=k, st=st, h=h, w=w: h1s[lo][k].ap()[:, st + h:st + h + w]),
                        k == 0, k == 1, None,
                    ))
            eng = "act" if di % 2 == 0 else "dve"
            emit_fill(
                data_waits, mms, eng,
                ("relu", 128,
                 (lambda lo=lo, m=m, st=st, nb=nb: h2s[lo][m].ap()[:, st:st + nb]),
                 (lambda lo=lo, m=m: b2s[lo].ap()[:, m:m + 1]),
                 ("h2a" if eng == "act" else "h2v", lo),
                 [(wsem[lo]["b2"], 16)]),
            )
            di += 1
        l2_thr[(lo, p)] = (prog_count[("h2a", lo)], prog_count[("h2v", lo)])

    def l3_dep_pair(lo, st_t, nb):
        """Index of the last L2 pair whose columns the block touches."""
        C, P = Cs[lo], Ps[lo]
        hi = min(2 * P + st_t + nb, C)
        dep = 0
        for pi, (pst, pnb) in enumerate(pairs[lo]):
            if pst < hi:
                dep = pi
        return dep

    def emit_l3(lo, bi):
        nonlocal di
        C, P = Cs[lo], Ps[lo]
        st_t, nb = l3blocks[lo][bi]
        dep = l3_dep_pair(lo, st_t, nb)
        na, nv = l2_thr[(lo, dep)]
        data_waits = [(wsem[lo]["w3"], 16)]
        if na:
            data_waits.append((prog[("h2a", lo)], na))
        if nv:
            data_waits.append((prog[("h2v", lo)], nv))
        mms = []
        for h, w in _halves(nb):
            for j in range(3):
                base = j * P + st_t + h
                wj = min(w, max(0, C - base))
                if wj <= 0:
                    continue
                for k in range(2):
                    mms.append((
                        32 * j, A, h, wj,
                        (lambda lo=lo, k=k: w3s[lo].ap()[:, k, :]),
                        (lambda lo=lo, k=k, base=base, wj=wj: h2s[lo][k].ap()[:, base:base + wj]),
                        k == 0, k == 1, (0, 32 * j),
                    ))
        eng = "act" if di % 2 == 0 else "dve"
        cnt = emit_fill(
            data_waits, mms, eng,
            ("copy", 96,
             (lambda lo=lo, st_t=st_t, nb=nb: osb[lo].ap()[:, st_t:st_t + nb]),
             None,
             ("oa" if eng == "act" else "ov", lo),
             []),
        )
        di += 1
        issue = "sync" if (out_dma_counts["sync"] <= out_dma_counts["gps"]) else "gps"
        osem = prog[("oa" if eng == "act" else "ov", lo)]
        odsem = od_sync if issue == "sync" else od_gps
        dma_op = ("dma", [(osem, cnt)],
                  (lambda lo=lo, st_t=st_t, nb=nb: out_p[lo][0][:, st_t:st_t + nb]),
                  (lambda lo=lo, st_t=st_t, nb=nb: osb[lo].ap()[:, st_t:st_t + nb]),
                  odsem, 16)
        (sync_ops if issue == "sync" else gps_ops).append(dma_op)
        out_dma_counts[issue] += 1

    # Global software pipeline: L1 two pair-groups ahead of L2; L3 blocks
    # emitted as soon as their last L2 dependency is in the stream.
    l1q = [(lo, p) for lo in range(OPC) for p in range(len(pairs[lo]))]
    l2q = list(l1q)
    l3q = [(lo, bi) for lo in range(OPC) for bi in range(len(l3blocks[lo]))]
    l2_emitted = set()
    emit_l1(*l1q.pop(0))
    emit_l1(*l1q.pop(0))
    for (lo, p) in l2q:
        if l1q:
            emit_l1(*l1q.pop(0))
        emit_l2(lo, p)
        l2_emitted.add((lo, p))
        while l3q:
            blo, bbi = l3q[0]
            st_t, nb = l3blocks[blo][bbi]
            if (blo, l3_dep_pair(blo, st_t, nb)) in l2_emitted:
                emit_l3(*l3q.pop(0))
            else:
                break
    while l3q:
        emit_l3(*l3q.pop(0))

    # --- emit engine programs ---
    with nc.Block() as block:

        @block.gpsimd
        def _(eng):
            for op in gps_ops:
                _, waits, dst_fn, src_fn, sem, val = op
                for wsem_, wval in waits:
                    eng.wait_ge(wsem_, wval)
                eng.dma_start(out=dst_fn(), in_=src_fn()).then_inc(sem, val)
            if out_dma_counts["gps"]:
                eng.wait_ge(od_gps, 16 * out_dma_counts["gps"])

        @block.sync
        def _(eng):
            for op in sync_ops:
                _, waits, dst_fn, src_fn, sem, val = op
                for wsem_, wval in waits:
                    eng.wait_ge(wsem_, wval)
                eng.dma_start(out=dst_fn(), in_=src_fn()).then_inc(sem, val)
            if out_dma_counts["sync"]:
                eng.wait_ge(od_sync, 16 * out_dma_counts["sync"])

        @block.tensor
        def _(eng):
            for _i in range(N_WARM_BIG):
                nc.tensor.matmul(
                    pss[_i % 4].ap()[:, 0:512], dummy.ap()[:, 0:128],
                    dummy.ap()[:, 0:512], start=True, stop=True,
                )
            for _i in range(N_WARM_SMALL):
                nc.tensor.matmul(
                    pss[_i % 4].ap()[:, 0:128], dummy.ap()[:, 0:128],
                    dummy.ap()[:, 0:128], start=True, stop=True,
                )
            for waits, s, mms, fdsem in pe_ops:
                for wsem_, wval in waits:
                    eng.wait_ge(wsem_, wval)
                for j, (olo, orow, h, w, lhs_fn, rhs_fn, stt, stp, tp) in enumerate(mms):
                    inst = nc.tensor.matmul(
                        pss[s].ap()[olo:olo + orow, h:h + w],
                        lhs_fn(), rhs_fn(), start=stt, stop=stp,
                        tile_position=tp,
                    )
                    if j == len(mms) - 1:
                        inst.then_inc(fdsem, 1)

        @block.scalar
        def _(eng):
            for op in scalar_dma_ops:
                _, waits, dst_fn, src_fn, sem, val = op
                for wsem_, wval in waits:
                    eng.wait_ge(wsem_, wval)
                eng.dma_start(out=dst_fn(), in_=src_fn()).then_inc(sem, val)
            # dummy activation pulls the Relu ACT table load into the prologue
            nc.scalar.activation(dummy_o.ap()[:], dummy.ap()[:, 0:1], AF.Relu, bias=0.0)
            for waits, kind, s, rows, dst_fn, bias_fn, sem in act_ops:
                for wsem_, wval in waits:
                    eng.wait_ge(wsem_, wval)
                dst = dst_fn()
                nbv = dst.shape[-1]
                src = pss[s].ap()[:rows, :nbv]
                if kind == "relu":
                    inst = nc.scalar.activation(dst, src, AF.Relu, bias=bias_fn())
                else:
                    inst = nc.scalar.activation(dst, src, AF.Copy)
                inst.then_inc(sem, 1)

        @block.vector
        def _(eng):
            for waits, kind, s, rows, dst_fn, bias_fn, sem in dve_ops:
                for wsem_, wval in waits:
                    eng.wait_ge(wsem_, wval)
                dst = dst_fn()
                nbv = dst.shape[-1]
                src = pss[s].ap()[:rows, :nbv]
                if kind == "relu":
                    inst = nc.vector.tensor_scalar(
                        dst, src, bias_fn(), 0.0, ALU.add, ALU.max
                    )
                else:
                    inst = nc.vector.tensor_copy(dst, src)
                inst.then_inc(sem, 1)

    nc.compile()
    return nc


def _get_program(C0, C1):
    key = (C0, C1)
    if key not in _CACHE:
        _CACHE[key] = _build_v2(C0, C1)
    return _CACHE[key]


def _pad128(n):
    return max(128, (n + 127) // 128 * 128)


def _prep(inputs):
    obs = np.ascontiguousarray(np.asarray(inputs["obs"], dtype=np.float32))
    option = np.asarray(inputs["option"]).astype(np.int64, copy=False)
    W1 = np.asarray(inputs["W1"], dtype=np.float32)
    b1 = np.asarray(inputs["b1"], dtype=np.float32)
    W2 = np.asarray(inputs["W2"], dtype=np.float32)
    b2 = np.asarray(inputs["b2"], dtype=np.float32)
    W3 = np.asarray(inputs["W3"], dtype=np.float32)
    b3 = np.asarray(inputs["b3"], dtype=np.float32)

    order = np.argsort(option, kind="stable")
    sorted_opt = option[order]
    starts = np.searchsorted(sorted_opt, np.arange(OPT + 1))
    idx_per_opt = [order[starts[o]: starts[o + 1]] for o in range(OPT)]
    cnt = np.array([len(ix) for ix in idx_per_opt])

    # pair largest with smallest to balance per-core work
    by_cnt = np.argsort(-cnt, kind="stable")
    assign = [(int(by_cnt[i]), int(by_cnt[OPT - 1 - i])) for i in range(NCORES)]
    C0 = max(_pad128(cnt[a]) for a, _ in assign)
    C1 = max(_pad128(cnt[b]) for _, b in assign)
    Cs = [C0, C1]
    Ps = [(C + 2) // 3 for C in Cs]

    obs16 = obs.astype(np.float16)
    in_maps = []
    for core in range(NCORES):
        m = {}
        for lo, o in enumerate(assign[core]):
            C = Cs[lo]
            idx = idx_per_opt[o][:C]
            xt = np.zeros((1, OBS, C), np.float16)
            xt[0, :, : len(idx)] = obs16[idx].T
            m[f"xt{lo}"] = xt
            m[f"w1_{lo}"] = np.ascontiguousarray(W1[o][None].astype(np.float16))
            m[f"w2_{lo}"] = np.ascontiguousarray(
                W2[o].reshape(2, 128, H2).transpose(1, 0, 2)[None].astype(np.float16))
            m[f"w3_{lo}"] = np.ascontiguousarray(
                W3[o].reshape(2, 128, A).transpose(1, 0, 2)[None].astype(np.float16))
            m[f"b1_{lo}"] = np.ascontiguousarray(b1[o].reshape(2, 128).T[None])
            m[f"b2_{lo}"] = np.ascontiguousarray(b2[o].reshape(2, 128).T[None])
        in_maps.append(m)
    host = dict(obs=obs, W1=W1, b1=b1, W2=W2, b2=b2, W3=W3, b3=b3)
    return in_maps, idx_per_opt, assign, (C0, C1), host


def _unshard(results, idx_per_opt, assign, CC, host):
    C0, C1 = CC
    Cs = [C0, C1]
    Ps = [(C + 2) // 3 for C in Cs]
    out_full = np.empty((B, 1, A), np.float32)
    for core in range(NCORES):
        for lo, o in enumerate(assign[core]):
            C, P = Cs[lo], Ps[lo]
            res = results[core][f"out{lo}"][0]  # [96, P] fp16
            idx = idx_per_opt[o]
            n = min(len(idx), C)
            for j in range(3):
                lo_s = j * P
                hi_s = min(n, (j + 1) * P) if j < 2 else n
                if hi_s <= lo_s:
                    continue
                seg = res[32 * j: 32 * j + A, : hi_s - lo_s]
                out_full[idx[lo_s:hi_s], 0, :] = seg.T + host["b3"][o]
            if len(idx) > n:  # overflow beyond C: compute on host (rare/never)
                rows = host["obs"][idx[n:]]
                h = np.maximum(rows @ host["W1"][o] + host["b1"][o], 0.0)
                h = np.maximum(h @ host["W2"][o] + host["b2"][o], 0.0)
                out_full[idx[n:], 0, :] = h @ host["W3"][o] + host["b3"][o]
    return out_full


def run(inputs, trace=False, **spmd_kwargs):
    """Run the kernel; returns (output, BassKernelResults)."""
    from concourse.bass_utils import run_bass_kernel_spmd

    in_maps, idx_per_opt, assign, CC, host = _prep(inputs)
    nc = _get_program(*CC)
    try:
        br = run_bass_kernel_spmd(
            nc, in_maps, list(range(NCORES)), trace=trace, **spmd_kwargs
        )
    except Exception:
        # transient device/runtime hiccups have been observed once per
        # session; rebuild the program and retry once
        _CACHE.clear()
        nc = _get_program(*CC)
        br = run_bass_kernel_spmd(
            nc, in_maps, list(range(NCORES)), trace=trace, **spmd_kwargs
        )
    return _unshard(br.results, idx_per_opt, assign, CC, host), br


def kernel(**inputs):
    out, _ = run(inputs)
    return out


# revision 15
# speedup vs baseline: 1.1968x; 1.1968x over previous
"""DiscreteOptionActor Trainium2 kernel (v2).

Computes, for each sample b, logits = MLP_{option[b]}(obs[b]) where each of the
16 options has its own 3-layer MLP (128 -> 256 -> 256 -> 18, ReLU).

Strategy (MoE routing, option-parallel):
  - Host groups samples by option (argsort); the 16 options are paired
    largest-with-smallest and one pair is assigned per core, so per-core
    column counts are balanced. Only the selected option's trunk is
    computed (16x less compute than the dense reference).
  - Per (core, slot) the gathered rows are padded to C0/C1 columns
    (multiples of 128, global maxima over cores so the SPMD program has
    one shape) and stored feature-major [128, C] in fp16.
  - Device: L1/L2 run fp16 matmuls with 512-col moving chunks into a
    4-slot PSUM rotation; bias+ReLU drains alternate ACT/DVE writing
    fp16. L3 (M=18) uses 3x PE column tiling (tile_position=(0,32j)):
    three 18-row output strips computed concurrently in one pass over
    each third of the columns.
  - Warm-up: dummy matmuls bridge the DMA latency window so the PE HAM
    clock-gate reaches 8/8 (2.4 GHz) before real work, with no idle gap.
  - DMA: three rings (sync HWDGE, scalar HWDGE, gpsimd SWDGE) stream xt
    and weights; first xt chunk + W1 lead the sync ring so L1 starts
    ~9us into the NEFF.
  - Host scatters results back to original row order and adds b3.
"""

import numpy as np

B, OBS, OPT, H1, H2, A = 65536, 128, 16, 256, 256, 18
NCORES = 8
OPC = 2  # options per core

_CACHE = {}

N_WARM_BIG = 4   # 512-col dummy matmuls
N_WARM_SMALL = 8  # 128-col dummy matmuls


def _halves(nb):
    out = []
    h = 0
    while h < nb:
        w = min(512, nb - h)
        out.append((h, w))
        h += w
    return out


def _pairs_for(C):
    """L1/L2 column blocks: 512,512 first (early start), then 1024s."""
    out = []
    st = 0
    for nb in (512, 512):
        if st < C:
            nb = min(nb, C - st)
            out.append((st, nb))
            st += nb
    while st < C:
        nb = min(1024, C - st)
        out.append((st, nb))
        st += nb
    return out


def _l3_blocks_for(P):
    out = []
    st = 0
    while st < P:
        nb = min(1024, P - st)
        out.append((st, nb))
        st += nb
    return out


def _chunks_from_bounds(C, bounds):
    out = []
    st = 0
    for b in list(bounds) + [C]:
        b = min(b, C)
        if b > st:
            out.append((st, b - st))
            st = b
    return out


def _build_v2(C0, C1):
    import concourse.bass as bass
    import concourse.bacc as bacc
    import concourse.mybir as mybir

    f32 = mybir.dt.float32
    f16 = mybir.dt.bfloat16
    AF = mybir.ActivationFunctionType
    ALU = mybir.AluOpType

    Cs = [C0, C1]
    Ps = [(C + 2) // 3 for C in Cs]

    nc = bacc.Bacc(None, target_bir_lowering=False, debug=False)

    xt_p, w1_p, w2_p, w3_p, b1_p, b2_p, out_p = [], [], [], [], [], [], []
    for lo in range(OPC):
        C, P = Cs[lo], Ps[lo]
        xt_p.append(nc.declare_dram_parameter(f"xt{lo}", [1, OBS, C], f16, isOutput=False))
        w1_p.append(nc.declare_dram_parameter(f"w1_{lo}", [1, OBS, H1], f16, isOutput=False))
        w2_p.append(nc.declare_dram_parameter(f"w2_{lo}", [1, 128, 2, H2], f16, isOutput=False))
        w3_p.append(nc.declare_dram_parameter(f"w3_{lo}", [1, 128, 2, A], f16, isOutput=False))
        b1_p.append(nc.declare_dram_parameter(f"b1_{lo}", [1, 128, 2], f32, isOutput=False))
        b2_p.append(nc.declare_dram_parameter(f"b2_{lo}", [1, 128, 2], f32, isOutput=False))
        out_p.append(nc.declare_dram_parameter(f"out{lo}", [1, 96, P], f16, isOutput=True))

    # --- on-chip tensors ---
    xts = [nc.alloc_sbuf_tensor(f"xts{lo}", [OBS, Cs[lo]], f16) for lo in range(OPC)]
    w1s = [nc.alloc_sbuf_tensor(f"w1s{lo}", [OBS, H1], f16) for lo in range(OPC)]
    w2s = [nc.alloc_sbuf_tensor(f"w2s{lo}", [128, 2, H2], f16) for lo in range(OPC)]
    w3s = [nc.alloc_sbuf_tensor(f"w3s{lo}", [128, 2, A], f16) for lo in range(OPC)]
    b1s = [nc.alloc_sbuf_tensor(f"b1s{lo}", [128, 2], f32) for lo in range(OPC)]
    b2s = [nc.alloc_sbuf_tensor(f"b2s{lo}", [128, 2], f32) for lo in range(OPC)]
    h1s = [[nc.alloc_sbuf_tensor(f"h1_{lo}_{c}", [128, Cs[lo]], f16) for c in range(2)]
           for lo in range(OPC)]
    h2s = [[nc.alloc_sbuf_tensor(f"h2_{lo}_{m}", [128, Cs[lo]], f16) for m in range(2)]
           for lo in range(OPC)]
    osb = [nc.alloc_sbuf_tensor(f"osb{lo}", [96, Ps[lo]], f16) for lo in range(OPC)]
    dummy = nc.alloc_sbuf_tensor("warm_dummy", [128, 512], f16)
    dummy_o = nc.alloc_sbuf_tensor("warm_dummy_o", [128, 1], f32)

    pss = [nc.alloc_psum_tensor(f"ps{s}", [128, 1024], f32) for s in range(4)]

    # --- semaphores ---
    # o0: critical chunk (0,512) leads the sync ring alone; bulk follows
    # gated on the critical phase so its packets can't delay it.
    xchunks = [
        _chunks_from_bounds(Cs[0], [512, 2048, 3136]),
        _chunks_from_bounds(Cs[1], [1024, 2560]),
    ]
    xsem = [[nc.alloc_semaphore(f"x{lo}_{ci}") for ci in range(len(xchunks[lo]))]
            for lo in range(OPC)]
    wname = ["w1", "b1", "b2", "w2", "w3"]
    wsem = [{n: nc.alloc_semaphore(f"wt{lo}_{n}") for n in wname} for lo in range(OPC)]
    fd = [nc.alloc_semaphore(f"fd{s}") for s in range(4)]
    prog = {}
    for lo in range(OPC):
        for key in ("h1a", "h1v", "h2a", "h2v", "oa", "ov"):
            prog[(key, lo)] = nc.alloc_semaphore(f"{key}{lo}")
    od_sync = nc.alloc_semaphore("od_sync")
    od_gps = nc.alloc_semaphore("od_gps")

    # --- static schedule state ---
    pe_ops = []   # (waits, slot, mms, fdsem); mm = (out_lo, out_rows, h, w, lhs_fn, rhs_fn, start, stop, tile_pos)
    act_ops = []  # (waits, kind, slot, rows, dst_fn, bias_fn, sem)
    dve_ops = []
    sync_ops = []  # ("dma", waits, dst_fn, src_fn, sem, val)
    scalar_dma_ops = []
    gps_ops = []

    fill_count = [0, 0, 0, 0]
    slot_prev_drain = [None, None, None, None]
    fill_idx = 0
    prog_count = {k: 0 for k in prog}
    pe_last_wait = {}

    def pe_wait(waits, sem, val):
        key = sem.name if hasattr(sem, "name") else id(sem)
        if pe_last_wait.get(key, -1) < val:
            waits.append((sem, val))
            pe_last_wait[key] = val

    def emit_fill(data_waits, mms, drain_engine, drain_emit):
        nonlocal fill_idx
        s = fill_idx % 4
        fill_idx += 1
        waits = []
        if slot_prev_drain[s] is not None:
            sem, cnt = slot_prev_drain[s]
            pe_wait(waits, sem, cnt)
        for sem, val in data_waits:
            pe_wait(waits, sem, val)
        pe_ops.append((waits, s, mms, fd[s]))
        fill_count[s] += 1
        fd_thresh = fill_count[s]
        kind, rows, dst_fn, bias, psem_key, extra_waits = drain_emit
        sem = prog[psem_key]
        prog_count[psem_key] += 1
        cnt = prog_count[psem_key]
        drain_waits = [(fd[s], fd_thresh)] + extra_waits
        op = (drain_waits, kind, s, rows, dst_fn, bias, sem)
        if drain_engine == "act":
            act_ops.append(op)
        else:
            dve_ops.append(op)
        slot_prev_drain[s] = (sem, cnt)
        return cnt

    # --- DMA issue schedule ---
    def xdma(lo, ci):
        cst, cnb = xchunks[lo][ci]
        return ("dma", [],
                (lambda lo=lo, cst=cst, cnb=cnb: xts[lo].ap()[:, cst:cst + cnb]),
                (lambda lo=lo, cst=cst, cnb=cnb: xt_p[lo][0][:, cst:cst + cnb]),
                xsem[lo][ci], 16)

    def wdma(lo, n):
        src = {"w1": w1_p, "b1": b1_p, "b2": b2_p, "w2": w2_p, "w3": w3_p}[n]
        dst = {"w1": w1s, "b1": b1s, "b2": b2s, "w2": w2s, "w3": w3s}[n]
        return ("dma", [], (lambda lo=lo, dst=dst: dst[lo].ap()[:]),
                (lambda lo=lo, src=src: src[lo][0]), wsem[lo][n], 16)

    # Phase A (critical, sync ring alone): xt0 first chunk + w1/b1.
    # Phase B (bulk) is gated on phase A's last sem so its packets don't
    # round-robin-compete with the critical chunk on the 16 SDMA engines.
    gate = [(wsem[0]["b1"], 16)]
    sync_ops.append(xdma(0, 0))
    sync_ops.append(wdma(0, "w1"))
    sync_ops.append(wdma(0, "b1"))
    # sync ring phase B (same ring FIFO, no explicit gate needed)
    sync_ops.append(wdma(0, "b2"))
    sync_ops.append(xdma(0, 1))
    sync_ops.append(wdma(0, "w2"))
    sync_ops.append(wdma(0, "w3"))
    sync_ops.append(xdma(1, 0))
    sync_ops.append(wdma(1, "w1"))
    sync_ops.append(wdma(1, "b1"))
    sync_ops.append(wdma(1, "b2"))
    # scalar ring (gated)
    op = xdma(0, 2)
    scalar_dma_ops.append((op[0], gate, op[2], op[3], op[4], op[5]))
    scalar_dma_ops.append(xdma(1, 1))
    scalar_dma_ops.append(wdma(1, "w2"))
    scalar_dma_ops.append(wdma(1, "w3"))
    # gps ring (gated)
    op = xdma(0, 3)
    gps_ops.append((op[0], gate, op[2], op[3], op[4], op[5]))
    gps_ops.append(xdma(1, 2))

    # --- fills ---
    pairs = [_pairs_for(C) for C in Cs]
    l3blocks = [_l3_blocks_for(P) for P in Ps]
    di = 0
    l1_thr = {}
    l2_thr = {}
    out_dma_counts = {"sync": 0, "gps": 0}

    def chunk_hi(lo, st, nb):
        hi = 0
        for ci, (cst, cnb) in enumerate(xchunks[lo]):
            if cst < st + nb:
                hi = ci
        return hi

    def emit_l1(lo, p):
        nonlocal di
        st, nb = pairs[lo][p]
        for c in range(2):
            data_waits = [(xsem[lo][ci], 16) for ci in range(chunk_hi(lo, st, nb) + 1)]
            data_waits.append((wsem[lo]["w1"], 16))
            mms = []
            for h, w in _halves(nb):
                mms.append((
                    0, 128, h, w,
                    (lambda lo=lo, c=c: w1s[lo].ap()[:, c * 128:(c + 1) * 128]),
                    (lambda lo=lo, st=st, h=h, w=w: xts[lo].ap()[:, st + h:st + h + w]),
                    True, True, None,
                ))
            eng = "act" if di % 2 == 0 else "dve"
            emit_fill(
                data_waits, mms, eng,
                ("relu", 128,
                 (lambda lo=lo, c=c, st=st, nb=nb: h1s[lo][c].ap()[:, st:st + nb]),
                 (lambda lo=lo, c=c: b1s[lo].ap()[:, c:c + 1]),
                 ("h1a" if eng == "act" else "h1v", lo),
                 [(wsem[lo]["b1"], 16)]),
            )
            di += 1
        l1_thr[(lo, p)] = (prog_count[("h1a", lo)], prog_count[("h1v", lo)])

    def emit_l2(lo, p):
        nonlocal di
        st, nb = pairs[lo][p]
        na, nv = l1_thr[(lo, p)]
        for m in range(2):
            data_waits = [(wsem[lo]["w2"], 16)]
            if na:
                data_waits.append((prog[("h1a", lo)], na))
            if nv:
                data_waits.append((prog[("h1v", lo)], nv))
            mms = []
            for h, w in _halves(nb):
                for k in range(2):
                    mms.append((
                        0, 128, h, w,
                        (lambda lo=lo, k=k, m=m: w2s[lo].ap()[:, k, m * 128:(m + 1) * 128]),
                        (lambda lo=lo, k=k, st=st, h=h, w=w: h1s[lo][k].ap()[:, st + h:st + h + w]),
                        k == 0, k == 1, None,
                    ))
            eng = "act" if di % 2 == 0 else "dve"
            emit_fill(
                data_waits, mms, eng,
                ("relu", 128,
                 (lambda lo=lo, m=m, st=st, nb=nb: h2s[lo][m].ap()[:, st:st + nb]),
                 (lambda lo=lo, m=m: b2s[lo].ap()[:, m:m + 1]),
                 ("h2a" if eng == "act" else "h2v", lo),
                 [(wsem[lo]["b2"], 16)]),
            )
            di += 1
        l2_thr[(lo, p)] = (prog_count[("h2a", lo)], prog_count[("h2v", lo)])

    def l3_dep_pair(lo, st_t, nb):
        """Index of the last L2 pair whose columns the block touches."""
        C, P = Cs[lo], Ps[lo]
        hi = min(2 * P + st_t + nb, C)
        dep = 0
        for pi, (pst, pnb) in enumerate(pairs[lo]):
            if pst < hi:
                dep = pi
        return dep

    def emit_l3(lo, bi):
        nonlocal di
        C, P = Cs[lo], Ps[lo]
        st_t, nb = l3blocks[lo][bi]
        dep = l3_dep_pair(lo, st_t, nb)
        na, nv = l2_thr[(lo, dep)]
        data_waits = [(wsem[lo]["w3"], 16)]
        if na:
            data_waits.append((prog[("h2a", lo)], na))
        if nv:
            data_waits.append((prog[("h2v", lo)], nv))
        mms = []
        for h, w in _halves(nb):
            # k outer so the three col-group tiles stream concurrently:
            # [k0:j0,j1,j2, k1:j0,j1,j2] — k1 of tile j waits only on k0 of
            # tile j (same cells), so the three tiles overlap ~fully.
            for k in range(2):
                for j in range(3):
                    base = j * P + st_t + h
                    wj = min(w, max(0, C - base))
                    if wj <= 0:
                        continue
                    mms.append((
                        32 * j, A, h, wj,
                        (lambda lo=lo, k=k: w3s[lo].ap()[:, k, :]),
                        (lambda lo=lo, k=k, base=base, wj=wj: h2s[lo][k].ap()[:, base:base + wj]),
                        k == 0, k == 1, (0, 32 * j),
                    ))
        eng = "act" if di % 2 == 0 else "dve"
        cnt = emit_fill(
            data_waits, mms, eng,
            ("copy", 96,
             (lambda lo=lo, st_t=st_t, nb=nb: osb[lo].ap()[:, st_t:st_t + nb]),
             None,
             ("oa" if eng == "act" else "ov", lo),
             []),
        )
        di += 1
        issue = "sync" if (out_dma_counts["sync"] <= out_dma_counts["gps"]) else "gps"
        osem = prog[("oa" if eng == "act" else "ov", lo)]
        odsem = od_sync if issue == "sync" else od_gps
        dma_op = ("dma", [(osem, cnt)],
                  (lambda lo=lo, st_t=st_t, nb=nb: out_p[lo][0][:, st_t:st_t + nb]),
                  (lambda lo=lo, st_t=st_t, nb=nb: osb[lo].ap()[:, st_t:st_t + nb]),
                  odsem, 16)
        (sync_ops if issue == "sync" else gps_ops).append(dma_op)
        out_dma_counts[issue] += 1

    # Global software pipeline: L1 two pair-groups ahead of L2; L3 blocks
    # emitted as soon as their last L2 dependency is in the stream.
    l1q = [(lo, p) for lo in range(OPC) for p in range(len(pairs[lo]))]
    l2q = list(l1q)
    l3q = [(lo, bi) for lo in range(OPC) for bi in range(len(l3blocks[lo]))]
    l2_emitted = set()
    ready_l3 = []  # L3 blocks whose deps are in the stream; emitted one
    #              iteration later so the PE has work while drains catch up
    emit_l1(*l1q.pop(0))
    emit_l1(*l1q.pop(0))
    for (lo, p) in l2q:
        if l1q:
            emit_l1(*l1q.pop(0))
        emit_l2(lo, p)
        l2_emitted.add((lo, p))
        # emit blocks that became ready in a PREVIOUS iteration (so this
        # iteration's L1+L2 fills sit between the dep drains and the L3)
        while ready_l3:
            emit_l3(*ready_l3.pop(0))
        while l3q:
            blo, bbi = l3q[0]
            st_t, nb = l3blocks[blo][bbi]
            if (blo, l3_dep_pair(blo, st_t, nb)) in l2_emitted:
                ready_l3.append(l3q.pop(0))
            else:
                break
    for b in ready_l3 + l3q:
        emit_l3(*b)

    # --- emit engine programs ---
    with nc.Block(no_gpsimd_drain=True) as block:

        @block.gpsimd
        def _(eng):
            for op in gps_ops:
                _, waits, dst_fn, src_fn, sem, val = op
                for wsem_, wval in waits:
                    eng.wait_ge(wsem_, wval)
                eng.dma_start(out=dst_fn(), in_=src_fn()).then_inc(sem, val)
            if out_dma_counts["gps"]:
                eng.wait_ge(od_gps, 16 * out_dma_counts["gps"])

        @block.sync
        def _(eng):
            for op in sync_ops:
                _, waits, dst_fn, src_fn, sem, val = op
                for wsem_, wval in waits:
                    eng.wait_ge(wsem_, wval)
                eng.dma_start(out=dst_fn(), in_=src_fn()).then_inc(sem, val)
            if out_dma_counts["sync"]:
                eng.wait_ge(od_sync, 16 * out_dma_counts["sync"])

        @block.tensor
        def _(eng):
            for _i in range(N_WARM_BIG):
                nc.tensor.matmul(
                    pss[_i % 4].ap()[:, 0:512], dummy.ap()[:, 0:128],
                    dummy.ap()[:, 0:512], start=True, stop=True,
                )
            for _i in range(N_WARM_SMALL):
                nc.tensor.matmul(
                    pss[_i % 4].ap()[:, 0:128], dummy.ap()[:, 0:128],
                    dummy.ap()[:, 0:128], start=True, stop=True,
                )
            for waits, s, mms, fdsem in pe_ops:
                for wsem_, wval in waits:
                    eng.wait_ge(wsem_, wval)
                for j, (olo, orow, h, w, lhs_fn, rhs_fn, stt, stp, tp) in enumerate(mms):
                    inst = nc.tensor.matmul(
                        pss[s].ap()[olo:olo + orow, h:h + w],
                        lhs_fn(), rhs_fn(), start=stt, stop=stp,
                        tile_position=tp, skip_group_check=(tp is not None),
                    )
                    if j == len(mms) - 1:
                        inst.then_inc(fdsem, 1)

        @block.scalar
        def _(eng):
            for op in scalar_dma_ops:
                _, waits, dst_fn, src_fn, sem, val = op
                for wsem_, wval in waits:
                    eng.wait_ge(wsem_, wval)
                eng.dma_start(out=dst_fn(), in_=src_fn()).then_inc(sem, val)
            # dummy activation pulls the Relu ACT table load into the prologue
            nc.scalar.activation(dummy_o.ap()[:], dummy.ap()[:, 0:1], AF.Relu, bias=0.0)
            for waits, kind, s, rows, dst_fn, bias_fn, sem in act_ops:
                for wsem_, wval in waits:
                    eng.wait_ge(wsem_, wval)
                dst = dst_fn()
                nbv = dst.shape[-1]
                src = pss[s].ap()[:rows, :nbv]
                if kind == "relu":
                    inst = nc.scalar.activation(dst, src, AF.Relu, bias=bias_fn())
                else:
                    inst = nc.scalar.activation(dst, src, AF.Copy)
                inst.then_inc(sem, 1)

        @block.vector
        def _(eng):
            for waits, kind, s, rows, dst_fn, bias_fn, sem in dve_ops:
                for wsem_, wval in waits:
                    eng.wait_ge(wsem_, wval)
                dst = dst_fn()
                nbv = dst.shape[-1]
                src = pss[s].ap()[:rows, :nbv]
                if kind == "relu":
                    inst = nc.vector.tensor_scalar(
                        dst, src, bias_fn(), 0.0, ALU.add, ALU.max
                    )
                else:
                    inst = nc.vector.tensor_copy(dst, src)
                inst.then_inc(sem, 1)

    nc.compile()
    return nc


def _get_program(C0, C1):
    key = (C0, C1)
    if key not in _CACHE:
        _CACHE[key] = _build_v2(C0, C1)
    return _CACHE[key]


def _pad128(n):
    return max(128, (n + 127) // 128 * 128)


def _prep(inputs):
    obs = np.ascontiguousarray(np.asarray(inputs["obs"], dtype=np.float32))
    option = np.asarray(inputs["option"]).astype(np.int64, copy=False)
    W1 = np.asarray(inputs["W1"], dtype=np.float32)
    b1 = np.asarray(inputs["b1"], dtype=np.float32)
    W2 = np.asarray(inputs["W2"], dtype=np.float32)
    b2 = np.asarray(inputs["b2"], dtype=np.float32)
    W3 = np.asarray(inputs["W3"], dtype=np.float32)
    b3 = np.asarray(inputs["b3"], dtype=np.float32)

    order = np.argsort(option, kind="stable")
    sorted_opt = option[order]
    starts = np.searchsorted(sorted_opt, np.arange(OPT + 1))
    idx_per_opt = [order[starts[o]: starts[o + 1]] for o in range(OPT)]
    cnt = np.array([len(ix) for ix in idx_per_opt])

    # pair largest with smallest to balance per-core work
    by_cnt = np.argsort(-cnt, kind="stable")
    assign = [(int(by_cnt[i]), int(by_cnt[OPT - 1 - i])) for i in range(NCORES)]
    C0 = max(_pad128(cnt[a]) for a, _ in assign)
    C1 = max(_pad128(cnt[b]) for _, b in assign)
    Cs = [C0, C1]
    Ps = [(C + 2) // 3 for C in Cs]

    from ml_dtypes import bfloat16

    obs16 = obs.astype(bfloat16)
    in_maps = []
    for core in range(NCORES):
        m = {}
        for lo, o in enumerate(assign[core]):
            C = Cs[lo]
            idx = idx_per_opt[o][:C]
            xt = np.zeros((1, OBS, C), bfloat16)
            xt[0, :, : len(idx)] = obs16[idx].T
            m[f"xt{lo}"] = xt
            m[f"w1_{lo}"] = np.ascontiguousarray(W1[o][None].astype(bfloat16))
            m[f"w2_{lo}"] = np.ascontiguousarray(
                W2[o].reshape(2, 128, H2).transpose(1, 0, 2)[None].astype(bfloat16))
            m[f"w3_{lo}"] = np.ascontiguousarray(
                W3[o].reshape(2, 128, A).transpose(1, 0, 2)[None].astype(bfloat16))
            m[f"b1_{lo}"] = np.ascontiguousarray(b1[o].reshape(2, 128).T[None])
            m[f"b2_{lo}"] = np.ascontiguousarray(b2[o].reshape(2, 128).T[None])
        in_maps.append(m)
    host = dict(obs=obs, W1=W1, b1=b1, W2=W2, b2=b2, W3=W3, b3=b3)
    return in_maps, idx_per_opt, assign, (C0, C1), host


def _unshard(results, idx_per_opt, assign, CC, host):
    C0, C1 = CC
    Cs = [C0, C1]
    Ps = [(C + 2) // 3 for C in Cs]
    out_full = np.empty((B, 1, A), np.float32)
    for core in range(NCORES):
        for lo, o in enumerate(assign[core]):
            C, P = Cs[lo], Ps[lo]
            res = np.asarray(results[core][f"out{lo}"][0], dtype=np.float32)  # [96, P]
            idx = idx_per_opt[o]
            n = min(len(idx), C)
            for j in range(3):
                lo_s = j * P
                hi_s = min(n, (j + 1) * P) if j < 2 else n
                if hi_s <= lo_s:
                    continue
                seg = res[32 * j: 32 * j + A, : hi_s - lo_s]
                out_full[idx[lo_s:hi_s], 0, :] = seg.T + host["b3"][o]
            if len(idx) > n:  # overflow beyond C: compute on host (rare/never)
                rows = host["obs"][idx[n:]]
                h = np.maximum(rows @ host["W1"][o] + host["b1"][o], 0.0)
                h = np.maximum(h @ host["W2"][o] + host["b2"][o], 0.0)
                out_full[idx[n:], 0, :] = h @ host["W3"][o] + host["b3"][o]
    return out_full


def run(inputs, trace=False, **spmd_kwargs):
    """Run the kernel; returns (output, BassKernelResults)."""
    from concourse.bass_utils import run_bass_kernel_spmd

    in_maps, idx_per_opt, assign, CC, host = _prep(inputs)
    nc = _get_program(*CC)
    try:
        br = run_bass_kernel_spmd(
            nc, in_maps, list(range(NCORES)), trace=trace, **spmd_kwargs
        )
    except Exception:
        # transient device/runtime hiccups have been observed once per
        # session; rebuild the program and retry once
        _CACHE.clear()
        nc = _get_program(*CC)
        br = run_bass_kernel_spmd(
            nc, in_maps, list(range(NCORES)), trace=trace, **spmd_kwargs
        )
    return _unshard(br.results, idx_per_opt, assign, CC, host), br


def kernel(**inputs):
    out, _ = run(inputs)
    return out
